# revision 10
# baseline (speedup 1.0000x reference)
"""MultiHeadAttention (B=1, L=4096, D=768, H=12) on 8 trn2 NeuronCores.

Sharding: data-parallel over query positions (L/8 = 512 queries per core).
Each core computes the full K/V projections (replicated; cheaper than any
cross-core collective on this platform), its slice of the Q projection,
attention for all 12 heads over its 512 queries, and the output projection
for its slice. No collectives; host concatenates the 8 output shards.

All matmul operands are fp16 (host-cast; PE runs fp16 at full rate and
accumulates fp32 in PSUM). Layouts, per core:
  - kp.T [768, 4096] fp16 in SBUF: scores lhsT slices come straight from it.
  - scores computed transposed [kpos, q], head pairs sharing one
    [128, 1024] PSUM tile so one Exp covers both heads (amortizes the
    per-activation fixed cost); exp output feeds AV as the moving operand.
  - vp [4096, 12*65] fp16 spilled to DRAM with a ones column per head:
    AV psum row 64 accumulates the softmax denominator for free.
  - no max-subtraction in softmax (scores ~ N(0,1): exp cannot overflow),
    bk dropped entirely (constant along the softmax axis), bv and bo folded
    into one output-side bias cb = Wo @ bv + bo.
  - K/V projection emission is interleaved l-chunk by l-chunk so attention
    (which consumes kp.T/vp at 128-column granularity) pipelines into it.
"""

import numpy as np

import concourse.bacc as bacc
import concourse.tile as tile
import concourse.mybir as mybir
from concourse.bass_utils import run_bass_kernel_spmd

P = 128
D_MODEL = 768
NUM_HEADS = 12
D_K = 64
NE = D_MODEL // P  # 6 tiles of the model dim
HA = 65            # head cols + ones column
USE_PBCAST = False  # DVE 0-stride partition broadcast for the softmax recip

F32 = mybir.dt.float32
F16 = mybir.dt.float16
Act = mybir.ActivationFunctionType


def build_program(L, LQ, n_cores):
    """Build + compile the per-core Bass program.

    L: total sequence length (keys/values), LQ: queries per core.
    """
    KT = L // P    # kpos chunks of 128 (scores stationary / AV contraction)
    LC = L // 512  # 512-wide l chunks for the kp.T projection
    QT = LQ // P   # query tiles of 128

    nc = bacc.Bacc("TRN2", target_bir_lowering=False, debug=False,
                   num_devices=n_cores)

    qT = nc.dram_tensor("qT", [D_MODEL, LQ], F16, kind="ExternalInput").ap()
    kT = nc.dram_tensor("kT", [D_MODEL, L], F16, kind="ExternalInput").ap()
    vT = nc.dram_tensor("vT", [D_MODEL, L], F16, kind="ExternalInput").ap()
    WqT = nc.dram_tensor("WqT", [D_MODEL, D_MODEL], F16, kind="ExternalInput").ap()
    WkT = nc.dram_tensor("WkT", [D_MODEL, D_MODEL], F16, kind="ExternalInput").ap()
    WvT = nc.dram_tensor("WvT", [D_MODEL, D_MODEL], F16, kind="ExternalInput").ap()
    WoT = nc.dram_tensor("WoT", [D_MODEL, D_MODEL], F16, kind="ExternalInput").ap()
    bq_r = nc.dram_tensor("bq_r", [P, NE], F32, kind="ExternalInput").ap()
    cb_bc = nc.dram_tensor("cb_bc", [P, D_MODEL], F32, kind="ExternalInput").ap()
    ones64 = nc.dram_tensor("ones64", [1, 64], F32, kind="ExternalInput").ap()
    out = nc.dram_tensor("out", [LQ, D_MODEL], F32, kind="ExternalOutput").ap()

    with tile.TileContext(nc) as tc:
        with (
            tc.tile_pool(name="persist", bufs=1) as persist,
            tc.tile_pool(name="dram", bufs=1, space="DRAM") as dram,
            tc.tile_pool(name="kt", bufs=8) as kt_pool,     # kT moving tiles
            tc.tile_pool(name="vt", bufs=8) as vt_pool,     # vT stationary tiles
            tc.tile_pool(name="stage", bufs=3) as stage,    # psum->dram staging
            tc.tile_pool(name="vh", bufs=6) as vh_pool,     # AV stationary chunks
            tc.tile_pool(name="exp", bufs=3) as exp_pool,
            tc.tile_pool(name="small", bufs=2) as small,
            tc.tile_pool(name="outst", bufs=2) as outst,
            tc.tile_pool(name="psA", bufs=2, space="PSUM") as psA,  # 2 banks
            tc.tile_pool(name="psS", bufs=2, space="PSUM") as psS,  # 4 banks
            tc.tile_pool(name="psV", bufs=2, space="PSUM") as psV,  # 2 banks
        ):
            # ---- persistent SBUF tensors ----
            kpT_sb = persist.tile([P, NE, L], F16)           # kp.T
            qpT_sb = persist.tile([P, NE, LQ], F16)          # qp.T
            attnT_sb = persist.tile([P, NE, LQ], F16)        # normalized attn.T
            qT_sb = persist.tile([P, NE, LQ], F16)
            Wq_sb = persist.tile([P, NE, D_MODEL], F16)
            Wk_sb = persist.tile([P, NE, D_MODEL], F16)
            WvT_sb = persist.tile([P, NE, D_MODEL], F16)
            WoT_sb = persist.tile([P, NE, D_MODEL], F16)
            bq_sb = persist.tile([P, NE], F32)
            cb_sb = persist.tile([P, D_MODEL], F32)
            ones_sb = persist.tile([1, 64], F32)

            vp_d = dram.tile([L, NUM_HEADS * HA], F16)       # vp + ones cols

            def load_wT(dst, src):
                nc.sync.dma_start(
                    out=dst[:], in_=src.rearrange("(t p) e -> p t e", p=P))

            # emission order = SP issue order: load what the first matmuls
            # need first; WoT/cb/ones are only needed at the tail.
            nc.sync.dma_start(out=qT_sb[:], in_=qT.rearrange(
                "(t p) l -> p t l", p=P))
            load_wT(Wq_sb, WqT)
            nc.sync.dma_start(out=bq_sb[:], in_=bq_r)
            load_wT(Wk_sb, WkT)
            load_wT(WvT_sb, WvT)

            vp_aug = vp_d[:].rearrange("l (h m) -> l h m", m=HA)

            # ---- qp.T [e, lq] = sum_d WqT[d, e].T @ qT[d, lq], + bq ----
            for e in range(NE):
                ps = psA.tile([P, 512], F32, name="pa")
                for d in range(NE):
                    nc.tensor.matmul(
                        ps[:, :LQ],
                        Wq_sb[:, d, e * P:(e + 1) * P],
                        qT_sb[:, d, :],
                        start=(d == 0), stop=(d == NE - 1),
                    )
                nc.scalar.activation(
                    qpT_sb[:, e, :], ps[:, :LQ], Act.Identity,
                    bias=bq_sb[:, e:e + 1],
                )

            # ---- K/V projections, interleaved per 1024-wide l group ----
            # kp.T [e, l] (bk dropped: softmax-shift invariant);
            # vp [l, e] with bv folded into cb and a ones column per head.
            for g in range(L // 1024):
                kt_tiles, vt_tiles = [], []
                for d in range(NE):
                    t = kt_pool.tile([P, 1024], F16, tag="kt")
                    nc.sync.dma_start(
                        out=t[:],
                        in_=kT[d * P:(d + 1) * P, g * 1024:(g + 1) * 1024])
                    kt_tiles.append(t)
                    t = vt_pool.tile([P, 1024], F16, tag="vt")
                    nc.sync.dma_start(
                        out=t[:],
                        in_=vT[d * P:(d + 1) * P, g * 1024:(g + 1) * 1024])
                    vt_tiles.append(t)
                for e in range(NE):
                    ps = psS.tile([P, 1024], F32, name="sc")
                    for half in range(2):
                        sl = slice(half * 512, half * 512 + 512)
                        for d in range(NE):
                            nc.tensor.matmul(
                                ps[:, sl],
                                Wk_sb[:, d, e * P:(e + 1) * P],
                                kt_tiles[d][:, sl],
                                start=(d == 0), stop=(d == NE - 1),
                            )
                    nc.vector.tensor_copy(
                        out=kpT_sb[:, e, g * 1024:(g + 1) * 1024], in_=ps[:])
                for lt2 in range(4):
                    st = stage.tile([P, 2, NUM_HEADS, HA], F16, tag="st")
                    for j in range(2):
                        lt_loc = lt2 * 2 + j
                        ps = psS.tile([P, 1024], F32, name="sc")
                        ps1, ps2 = ps[:, 0:512], ps[:, 512:768]
                        for d in range(NE):
                            vsl = vt_tiles[d][:, lt_loc * P:(lt_loc + 1) * P]
                            nc.tensor.matmul(ps1, vsl, WvT_sb[:, d, 0:512],
                                             start=(d == 0), stop=(d == NE - 1))
                            nc.tensor.matmul(ps2, vsl, WvT_sb[:, d, 512:768],
                                             start=(d == 0), stop=(d == NE - 1))
                        nc.vector.tensor_copy(
                            out=st[:, j, :, 0:64],
                            in_=ps[:, :D_MODEL].rearrange(
                                "p (h m) -> p h m", m=64))
                        nc.vector.memset(st[:, j, :, 64:65], 1.0)
                    r0 = g * 1024 + lt2 * 256
                    nc.sync.dma_start(
                        out=vp_d[r0:r0 + 256, :].rearrange(
                            "(a p) m -> p a m", p=P),
                        in_=st[:].rearrange("p a h m -> p a (h m)"))

            load_wT(WoT_sb, WoT)
            nc.sync.dma_start(out=cb_sb[:], in_=cb_bc)
            nc.sync.dma_start(out=ones_sb[:], in_=ones64)

            # ---- attention, head-pair by head-pair ----
            HKT = KT // 2
            for hp in range(NUM_HEADS // 2):
                h0, h1 = 2 * hp, 2 * hp + 1
                et = h0 // 2
                q0 = qpT_sb[0:64, et, :]
                q1 = qpT_sb[64:128, et, :]
                av0 = psV.tile([HA, 512], F32, name="av")
                av1 = psV.tile([HA, 512], F32, name="av")
                vhs = {}
                for half in range(2):
                    for i, h in enumerate((h0, h1)):
                        t = vh_pool.tile([P, HKT, HA], F16, tag="vh")
                        rows = vp_aug[half * HKT * P:(half + 1) * HKT * P, h, :]
                        nc.sync.dma_start(
                            out=t[:], in_=rows.rearrange("(c p) m -> p c m", p=P))
                        vhs[(half, i)] = t
                for c in range(KT):
                    half, cl = c // HKT, c % HKT
                    ps_s = psS.tile([P, 1024], F32, name="sc")
                    nc.tensor.matmul(
                        ps_s[:, 0:LQ],
                        kpT_sb[0:64, et, c * P:(c + 1) * P], q0,
                        start=True, stop=True)
                    nc.tensor.matmul(
                        ps_s[:, 512:512 + LQ],
                        kpT_sb[64:128, et, c * P:(c + 1) * P], q1,
                        start=True, stop=True)
                    ex = exp_pool.tile([P, 1024], F16, tag="exp")
                    nc.scalar.activation(ex[:], ps_s[:], Act.Exp, scale=0.125)
                    nc.tensor.matmul(av0[:, :LQ], vhs[(half, 0)][:, cl, :],
                                     ex[:, 0:LQ],
                                     start=(c == 0), stop=(c == KT - 1))
                    nc.tensor.matmul(av1[:, :LQ], vhs[(half, 1)][:, cl, :],
                                     ex[:, 512:512 + LQ],
                                     start=(c == 0), stop=(c == KT - 1))
                for h, av in ((h0, av0), (h1, av1)):
                    pr = (h % 2) * 64
                    # copy PSUM out immediately so the AV slot frees for the
                    # next pair; the normalize tail works from SBUF
                    av_s = small.tile([HA, 512], F32, tag="avs")
                    nc.vector.tensor_copy(out=av_s[:, :LQ], in_=av[:, :LQ])
                    recip = small.tile([1, 512], F32, tag="recip")
                    nc.vector.reciprocal(out=recip[:, :LQ],
                                         in_=av_s[64:65, :LQ])
                    if USE_PBCAST:
                        rb = recip[:, :LQ].partition_broadcast(64)[:, 0, :]
                        nc.vector.tensor_tensor(
                            out=attnT_sb[pr:pr + 64, et, :],
                            in0=av_s[0:64, :LQ], in1=rb,
                            op=mybir.AluOpType.mult,
                        )
                    else:
                        ps_bc = psA.tile([64, 512], F32, name="pa")
                        nc.tensor.matmul(ps_bc[:, :LQ], ones_sb[:],
                                         recip[:, :LQ], start=True, stop=True)
                        rbc = small.tile([64, 512], F32, tag="rbc")
                        nc.vector.tensor_copy(out=rbc[:, :LQ], in_=ps_bc[:, :LQ])
                        nc.vector.tensor_tensor(
                            out=attnT_sb[pr:pr + 64, et, :],
                            in0=av_s[0:64, :LQ], in1=rbc[:, :LQ],
                            op=mybir.AluOpType.mult,
                        )

            # ---- out[q, e] = attnT.T @ WoT + cb ----
            for qt in range(QT):
                ps1 = psA.tile([P, 512], F32, name="pa")
                ps2 = psA.tile([P, 512], F32, name="pa")[:, :256]
                for d in range(NE):
                    lhs = attnT_sb[:, d, qt * P:(qt + 1) * P]
                    nc.tensor.matmul(ps1[:], lhs, WoT_sb[:, d, 0:512],
                                     start=(d == 0), stop=(d == NE - 1))
                    nc.tensor.matmul(ps2[:], lhs, WoT_sb[:, d, 512:768],
                                     start=(d == 0), stop=(d == NE - 1))
                ot = outst.tile([P, D_MODEL], F32, tag="ot")
                nc.vector.tensor_tensor(out=ot[:, 0:512], in0=ps1[:],
                                        in1=cb_sb[:, 0:512],
                                        op=mybir.AluOpType.add)
                nc.vector.tensor_tensor(out=ot[:, 512:768], in0=ps2[:],
                                        in1=cb_sb[:, 512:768],
                                        op=mybir.AluOpType.add)
                nc.sync.dma_start(out=out[qt * P:(qt + 1) * P, :], in_=ot[:])

    nc.compile()
    return nc


def make_in_maps(q, k, v, Wq, bq, Wk, bk, Wv, bv, Wo, bo, L, LQ, n_cores):
    f32, f16 = np.float32, np.float16
    qT_full = np.ascontiguousarray(q[0].T, dtype=f16)       # [768, L]
    kT_full = np.ascontiguousarray(k[0].T, dtype=f16)
    vT_full = np.ascontiguousarray(v[0].T, dtype=f16)
    WqT = np.ascontiguousarray(np.asarray(Wq, f32).T.astype(f16))
    WkT = np.ascontiguousarray(np.asarray(Wk, f32).T.astype(f16))
    WvT = np.ascontiguousarray(np.asarray(Wv, f32).T.astype(f16))
    WoT = np.ascontiguousarray(np.asarray(Wo, f32).T.astype(f16))
    bq_r = np.ascontiguousarray(np.asarray(bq, f32).reshape(NE, P).T)
    cb = np.asarray(Wo, f32) @ np.asarray(bv, f32) + np.asarray(bo, f32)
    cb_bc = np.ascontiguousarray(np.broadcast_to(cb, (P, D_MODEL)))
    shared = dict(kT=kT_full, vT=vT_full, WqT=WqT, WkT=WkT, WvT=WvT,
                  WoT=WoT, bq_r=bq_r, cb_bc=cb_bc,
                  ones64=np.ones((1, 64), f32))
    return [
        {"qT": np.ascontiguousarray(qT_full[:, c * LQ:(c + 1) * LQ]), **shared}
        for c in range(n_cores)
    ]


_PROGRAM_CACHE = {}


def get_program(L, LQ, n_cores):
    key = (L, LQ, n_cores)
    if key not in _PROGRAM_CACHE:
        _PROGRAM_CACHE[key] = build_program(L, LQ, n_cores)
    return _PROGRAM_CACHE[key]


def kernel(q, k, v, Wq, bq, Wk, bk, Wv, bv, Wo, bo):
    B, L, _ = q.shape
    assert B == 1
    n_cores = 8
    LQ = L // n_cores
    nc = get_program(L, LQ, n_cores)
    in_maps = make_in_maps(q, k, v, Wq, bq, Wk, bk, Wv, bv, Wo, bo,
                           L, LQ, n_cores)
    res = run_bass_kernel_spmd(nc, in_maps, core_ids=list(range(n_cores)))
    full = np.concatenate([res.results[c]["out"] for c in range(n_cores)], axis=0)
    return full[None].astype(np.float32)


# revision 16
# speedup vs baseline: 1.0052x; 1.0052x over previous
"""MultiHeadAttention (B=1, L=4096, D=768, H=12) on 8 trn2 NeuronCores.

Sharding: data-parallel over query positions (L/8 = 512 queries per core).
Each core computes the full K/V projections (replicated; cheaper than any
cross-core collective on this platform), its slice of the Q projection,
attention for all 12 heads over its 512 queries, and the output projection
for its slice. No collectives; host concatenates the 8 output shards.

All matmul operands are fp16 (host-cast; PE runs fp16 at full rate and
accumulates fp32 in PSUM). Layouts, per core:
  - kp.T [768, 4096] fp16 in SBUF: scores lhsT slices come straight from it.
  - scores computed transposed [kpos, q], head pairs sharing one
    [128, 1024] PSUM tile so one Exp covers both heads (amortizes the
    per-activation fixed cost); exp output feeds AV as the moving operand.
  - vp [4096, 12*65] fp16 spilled to DRAM with a ones column per head:
    AV psum row 64 accumulates the softmax denominator for free.
  - no max-subtraction in softmax (scores ~ N(0,1): exp cannot overflow),
    bk dropped entirely (constant along the softmax axis), bv and bo folded
    into one output-side bias cb = Wo @ bv + bo.
  - K/V projection emission is interleaved l-chunk by l-chunk so attention
    (which consumes kp.T/vp at 128-column granularity) pipelines into it.
"""

import numpy as np

import concourse.bacc as bacc
import concourse.tile as tile
import concourse.mybir as mybir
from concourse.bass_utils import run_bass_kernel_spmd

P = 128
D_MODEL = 768
NUM_HEADS = 12
D_K = 64
NE = D_MODEL // P  # 6 tiles of the model dim
HA = 65            # head cols + ones column
USE_PBCAST = False  # DVE 0-stride partition broadcast for the softmax recip

F32 = mybir.dt.float32
F16 = mybir.dt.float16
Act = mybir.ActivationFunctionType


def build_program(L, LQ, n_cores):
    """Build + compile the per-core Bass program.

    L: total sequence length (keys/values), LQ: queries per core.
    """
    KT = L // P    # kpos chunks of 128 (scores stationary / AV contraction)
    LC = L // 512  # 512-wide l chunks for the kp.T projection
    QT = LQ // P   # query tiles of 128

    nc = bacc.Bacc("TRN2", target_bir_lowering=False, debug=False,
                   num_devices=n_cores)

    qT = nc.dram_tensor("qT", [D_MODEL, LQ], F16, kind="ExternalInput").ap()
    kT = nc.dram_tensor("kT", [D_MODEL, L], F16, kind="ExternalInput").ap()
    vT = nc.dram_tensor("vT", [D_MODEL, L], F16, kind="ExternalInput").ap()
    WqT = nc.dram_tensor("WqT", [D_MODEL, D_MODEL], F16, kind="ExternalInput").ap()
    WkT = nc.dram_tensor("WkT", [D_MODEL, D_MODEL], F16, kind="ExternalInput").ap()
    WvT = nc.dram_tensor("WvT", [D_MODEL, D_MODEL], F16, kind="ExternalInput").ap()
    WoT = nc.dram_tensor("WoT", [D_MODEL, D_MODEL], F16, kind="ExternalInput").ap()
    bq_r = nc.dram_tensor("bq_r", [P, NE], F32, kind="ExternalInput").ap()
    cb_bc = nc.dram_tensor("cb_bc", [P, D_MODEL], F32, kind="ExternalInput").ap()
    ones64 = nc.dram_tensor("ones64", [1, 64], F32, kind="ExternalInput").ap()
    out = nc.dram_tensor("out", [LQ, D_MODEL], F32, kind="ExternalOutput").ap()

    with tile.TileContext(nc) as tc:
        with (
            tc.tile_pool(name="persist", bufs=1) as persist,
            tc.tile_pool(name="dram", bufs=1, space="DRAM") as dram,
            tc.tile_pool(name="kt", bufs=8) as kt_pool,     # kT moving tiles
            tc.tile_pool(name="vt", bufs=8) as vt_pool,     # vT stationary tiles
            tc.tile_pool(name="stage", bufs=3) as stage,    # psum->dram staging
            tc.tile_pool(name="vh", bufs=6) as vh_pool,     # AV stationary chunks
            tc.tile_pool(name="exp", bufs=3) as exp_pool,
            tc.tile_pool(name="small", bufs=2) as small,
            tc.tile_pool(name="outst", bufs=2) as outst,
            tc.tile_pool(name="psS", bufs=3, space="PSUM") as psS,  # 6 banks
            tc.tile_pool(name="psV", bufs=2, space="PSUM") as psV,  # 2 banks
        ):
            # ---- persistent SBUF tensors ----
            kpT_sb = persist.tile([P, NE, L], F16)           # kp.T
            qpT_sb = persist.tile([P, NE, LQ], F16)          # qp.T
            attnT_sb = persist.tile([P, NE, LQ], F16)        # normalized attn.T
            qT_sb = persist.tile([P, NE, LQ], F16)
            Wq_sb = persist.tile([P, NE, D_MODEL], F16)
            Wk_sb = persist.tile([P, NE, D_MODEL], F16)
            WvT_sb = persist.tile([P, NE, D_MODEL], F16)
            WoT_sb = persist.tile([P, NE, D_MODEL], F16)
            bq_sb = persist.tile([P, NE], F32)
            cb_sb = persist.tile([P, D_MODEL], F32)
            ones_sb = persist.tile([1, 64], F32)

            vp_d = dram.tile([L, NUM_HEADS * HA], F16)       # vp + ones cols

            def load_wT(dst, src):
                nc.sync.dma_start(
                    out=dst[:], in_=src.rearrange("(t p) e -> p t e", p=P))

            # emission order = SP issue order: load what the first matmuls
            # need first; WoT/cb/ones are only needed at the tail.
            nc.sync.dma_start(out=qT_sb[:], in_=qT.rearrange(
                "(t p) l -> p t l", p=P))
            load_wT(Wq_sb, WqT)
            nc.sync.dma_start(out=bq_sb[:], in_=bq_r)
            load_wT(Wk_sb, WkT)
            load_wT(WvT_sb, WvT)

            vp_aug = vp_d[:].rearrange("l (h m) -> l h m", m=HA)

            # ---- qp.T [e, lq] = sum_d WqT[d, e].T @ qT[d, lq], + bq ----
            for e in range(NE):
                ps = psS.tile([P, 1024], F32, name="sc")[:, :512]
                for d in range(NE):
                    nc.tensor.matmul(
                        ps[:, :LQ],
                        Wq_sb[:, d, e * P:(e + 1) * P],
                        qT_sb[:, d, :],
                        start=(d == 0), stop=(d == NE - 1),
                    )
                nc.scalar.activation(
                    qpT_sb[:, e, :], ps[:, :LQ], Act.Identity,
                    bias=bq_sb[:, e:e + 1],
                )

            # ---- K/V projections, interleaved per 1024-wide l group ----
            # kp.T [e, l] (bk dropped: softmax-shift invariant);
            # vp [l, e] with bv folded into cb and a ones column per head.
            for g in range(L // 1024):
                kt_tiles, vt_tiles = [], []
                for d in range(NE):
                    t = kt_pool.tile([P, 1024], F16, tag="kt")
                    nc.sync.dma_start(
                        out=t[:],
                        in_=kT[d * P:(d + 1) * P, g * 1024:(g + 1) * 1024])
                    kt_tiles.append(t)
                    t = vt_pool.tile([P, 1024], F16, tag="vt")
                    nc.sync.dma_start(
                        out=t[:],
                        in_=vT[d * P:(d + 1) * P, g * 1024:(g + 1) * 1024])
                    vt_tiles.append(t)
                for e in range(NE):
                    ps = psS.tile([P, 1024], F32, name="sc")
                    for half in range(2):
                        sl = slice(half * 512, half * 512 + 512)
                        for d in range(NE):
                            nc.tensor.matmul(
                                ps[:, sl],
                                Wk_sb[:, d, e * P:(e + 1) * P],
                                kt_tiles[d][:, sl],
                                start=(d == 0), stop=(d == NE - 1),
                            )
                    nc.vector.tensor_copy(
                        out=kpT_sb[:, e, g * 1024:(g + 1) * 1024], in_=ps[:])
                for lt2 in range(4):
                    st = stage.tile([P, 2, NUM_HEADS, HA], F16, tag="st")
                    for j in range(2):
                        lt_loc = lt2 * 2 + j
                        ps = psS.tile([P, 1024], F32, name="sc")
                        ps1, ps2 = ps[:, 0:512], ps[:, 512:768]
                        for d in range(NE):
                            vsl = vt_tiles[d][:, lt_loc * P:(lt_loc + 1) * P]
                            nc.tensor.matmul(ps1, vsl, WvT_sb[:, d, 0:512],
                                             start=(d == 0), stop=(d == NE - 1))
                            nc.tensor.matmul(ps2, vsl, WvT_sb[:, d, 512:768],
                                             start=(d == 0), stop=(d == NE - 1))
                        nc.vector.tensor_copy(
                            out=st[:, j, :, 0:64],
                            in_=ps[:, :D_MODEL].rearrange(
                                "p (h m) -> p h m", m=64))
                        nc.vector.memset(st[:, j, :, 64:65], 1.0)
                    r0 = g * 1024 + lt2 * 256
                    nc.sync.dma_start(
                        out=vp_d[r0:r0 + 256, :].rearrange(
                            "(a p) m -> p a m", p=P),
                        in_=st[:].rearrange("p a h m -> p a (h m)"))

            load_wT(WoT_sb, WoT)
            nc.sync.dma_start(out=cb_sb[:], in_=cb_bc)
            nc.sync.dma_start(out=ones_sb[:], in_=ones64)

            # ---- attention, head-pair by head-pair ----
            HKT = KT // 2
            for hp in range(NUM_HEADS // 2):
                h0, h1 = 2 * hp, 2 * hp + 1
                et = h0 // 2
                q0 = qpT_sb[0:64, et, :]
                q1 = qpT_sb[64:128, et, :]
                av0 = psV.tile([HA, 512], F32, name="av")
                av1 = psV.tile([HA, 512], F32, name="av")
                vhs = {}
                for half in range(2):
                    for i, h in enumerate((h0, h1)):
                        t = vh_pool.tile([P, HKT, HA], F16, tag="vh")
                        rows = vp_aug[half * HKT * P:(half + 1) * HKT * P, h, :]
                        nc.sync.dma_start(
                            out=t[:], in_=rows.rearrange("(c p) m -> p c m", p=P))
                        vhs[(half, i)] = t
                for c in range(KT):
                    half, cl = c // HKT, c % HKT
                    ps_s = psS.tile([P, 1024], F32, name="sc")
                    nc.tensor.matmul(
                        ps_s[:, 0:LQ],
                        kpT_sb[0:64, et, c * P:(c + 1) * P], q0,
                        start=True, stop=True)
                    nc.tensor.matmul(
                        ps_s[:, 512:512 + LQ],
                        kpT_sb[64:128, et, c * P:(c + 1) * P], q1,
                        start=True, stop=True)
                    ex = exp_pool.tile([P, 1024], F16, tag="exp")
                    nc.scalar.activation(ex[:], ps_s[:], Act.Exp, scale=0.125)
                    nc.tensor.matmul(av0[:, :LQ], vhs[(half, 0)][:, cl, :],
                                     ex[:, 0:LQ],
                                     start=(c == 0), stop=(c == KT - 1))
                    nc.tensor.matmul(av1[:, :LQ], vhs[(half, 1)][:, cl, :],
                                     ex[:, 512:512 + LQ],
                                     start=(c == 0), stop=(c == KT - 1))
                for h, av in ((h0, av0), (h1, av1)):
                    pr = (h % 2) * 64
                    # copy PSUM out immediately so the AV slot frees for the
                    # next pair; the normalize tail works from SBUF
                    av_s = small.tile([HA, 512], F32, tag="avs")
                    nc.vector.tensor_copy(out=av_s[:, :LQ], in_=av[:, :LQ])
                    recip = small.tile([1, 512], F32, tag="recip")
                    nc.vector.reciprocal(out=recip[:, :LQ],
                                         in_=av_s[64:65, :LQ])
                    if USE_PBCAST:
                        rb = recip[:, :LQ].partition_broadcast(64)[:, 0, :]
                        nc.vector.tensor_tensor(
                            out=attnT_sb[pr:pr + 64, et, :],
                            in0=av_s[0:64, :LQ], in1=rb,
                            op=mybir.AluOpType.mult,
                        )
                    else:
                        ps_bc = psS.tile([P, 1024], F32, name="sc")[0:64, 0:512]
                        nc.tensor.matmul(ps_bc[:, :LQ], ones_sb[:],
                                         recip[:, :LQ], start=True, stop=True)
                        rbc = small.tile([64, 512], F32, tag="rbc")
                        nc.vector.tensor_copy(out=rbc[:, :LQ], in_=ps_bc[:, :LQ])
                        nc.vector.tensor_tensor(
                            out=attnT_sb[pr:pr + 64, et, :],
                            in0=av_s[0:64, :LQ], in1=rbc[:, :LQ],
                            op=mybir.AluOpType.mult,
                        )

            # ---- out[q, e] = attnT.T @ WoT + cb ----
            for qt in range(QT):
                pso = psS.tile([P, 1024], F32, name="sc")
                ps1, ps2 = pso[:, 0:512], pso[:, 512:768]
                for d in range(NE):
                    lhs = attnT_sb[:, d, qt * P:(qt + 1) * P]
                    nc.tensor.matmul(ps1[:], lhs, WoT_sb[:, d, 0:512],
                                     start=(d == 0), stop=(d == NE - 1))
                    nc.tensor.matmul(ps2[:], lhs, WoT_sb[:, d, 512:768],
                                     start=(d == 0), stop=(d == NE - 1))
                ot = outst.tile([P, D_MODEL], F32, tag="ot")
                nc.vector.tensor_tensor(out=ot[:, 0:512], in0=ps1[:],
                                        in1=cb_sb[:, 0:512],
                                        op=mybir.AluOpType.add)
                nc.vector.tensor_tensor(out=ot[:, 512:768], in0=ps2[:],
                                        in1=cb_sb[:, 512:768],
                                        op=mybir.AluOpType.add)
                nc.sync.dma_start(out=out[qt * P:(qt + 1) * P, :], in_=ot[:])

    nc.compile()
    return nc


def make_in_maps(q, k, v, Wq, bq, Wk, bk, Wv, bv, Wo, bo, L, LQ, n_cores):
    f32, f16 = np.float32, np.float16
    qT_full = np.ascontiguousarray(q[0].T, dtype=f16)       # [768, L]
    kT_full = np.ascontiguousarray(k[0].T, dtype=f16)
    vT_full = np.ascontiguousarray(v[0].T, dtype=f16)
    WqT = np.ascontiguousarray(np.asarray(Wq, f32).T.astype(f16))
    WkT = np.ascontiguousarray(np.asarray(Wk, f32).T.astype(f16))
    WvT = np.ascontiguousarray(np.asarray(Wv, f32).T.astype(f16))
    WoT = np.ascontiguousarray(np.asarray(Wo, f32).T.astype(f16))
    bq_r = np.ascontiguousarray(np.asarray(bq, f32).reshape(NE, P).T)
    cb = np.asarray(Wo, f32) @ np.asarray(bv, f32) + np.asarray(bo, f32)
    cb_bc = np.ascontiguousarray(np.broadcast_to(cb, (P, D_MODEL)))
    shared = dict(kT=kT_full, vT=vT_full, WqT=WqT, WkT=WkT, WvT=WvT,
                  WoT=WoT, bq_r=bq_r, cb_bc=cb_bc,
                  ones64=np.ones((1, 64), f32))
    return [
        {"qT": np.ascontiguousarray(qT_full[:, c * LQ:(c + 1) * LQ]), **shared}
        for c in range(n_cores)
    ]


_PROGRAM_CACHE = {}


def get_program(L, LQ, n_cores):
    key = (L, LQ, n_cores)
    if key not in _PROGRAM_CACHE:
        _PROGRAM_CACHE[key] = build_program(L, LQ, n_cores)
    return _PROGRAM_CACHE[key]


def kernel(q, k, v, Wq, bq, Wk, bk, Wv, bv, Wo, bo):
    B, L, _ = q.shape
    assert B == 1
    n_cores = 8
    LQ = L // n_cores
    nc = get_program(L, LQ, n_cores)
    in_maps = make_in_maps(q, k, v, Wq, bq, Wk, bk, Wv, bv, Wo, bo,
                           L, LQ, n_cores)
    res = run_bass_kernel_spmd(nc, in_maps, core_ids=list(range(n_cores)))
    full = np.concatenate([res.results[c]["out"] for c in range(n_cores)], axis=0)
    return full[None].astype(np.float32)


# revision 23
# speedup vs baseline: 1.0734x; 1.0679x over previous
"""MultiHeadAttention (B=1, L=4096, D=768, H=12) on 8 trn2 NeuronCores.

Sharding: data-parallel over query positions (L/8 = 512 queries per core).
Each core computes the full K/V projections (replicated; cheaper than any
cross-core collective on this platform), its slice of the Q projection,
attention for all 12 heads over its 512 queries, and the output projection
for its slice. No collectives; host concatenates the 8 output shards.

All matmul operands are fp16 (host-cast; PE runs fp16 at full rate and
accumulates fp32 in PSUM). Layouts, per core:
  - kp.T [768, 4096] fp16 in SBUF: scores lhsT slices come straight from it.
  - scores computed transposed [kpos, q], head pairs sharing one
    [128, 1024] PSUM tile so one Exp covers both heads (amortizes the
    per-activation fixed cost); exp output feeds AV as the moving operand.
  - vp [4096, 12*65] fp16 spilled to DRAM with a ones column per head:
    AV psum row 64 accumulates the softmax denominator for free.
  - no max-subtraction in softmax (scores ~ N(0,1): exp cannot overflow),
    bk dropped entirely (constant along the softmax axis), bv and bo folded
    into one output-side bias cb = Wo @ bv + bo.
  - K/V projection emission is interleaved l-chunk by l-chunk so attention
    (which consumes kp.T/vp at 128-column granularity) pipelines into it.
"""

import numpy as np

import concourse.bacc as bacc
import concourse.tile as tile
import concourse.mybir as mybir
from concourse.bass_utils import run_bass_kernel_spmd

P = 128
D_MODEL = 768
NUM_HEADS = 12
D_K = 64
NE = D_MODEL // P  # 6 tiles of the model dim
HA = 65            # head cols + ones column
USE_PBCAST = False  # DVE 0-stride partition broadcast for the softmax recip

F32 = mybir.dt.float32
F16 = mybir.dt.float16
Act = mybir.ActivationFunctionType


def build_program(L, LQ, n_cores):
    """Build + compile the per-core Bass program.

    L: total sequence length (keys/values), LQ: queries per core.
    """
    KT = L // P    # kpos chunks of 128 (scores stationary / AV contraction)
    LC = L // 512  # 512-wide l chunks for the kp.T projection
    QT = LQ // P   # query tiles of 128

    nc = bacc.Bacc("TRN2", target_bir_lowering=False, debug=False,
                   num_devices=n_cores)

    qT = nc.dram_tensor("qT", [D_MODEL, LQ], F16, kind="ExternalInput").ap()
    kT = nc.dram_tensor("kT", [D_MODEL, L], F16, kind="ExternalInput").ap()
    vT = nc.dram_tensor("vT", [D_MODEL, L], F16, kind="ExternalInput").ap()
    WqT = nc.dram_tensor("WqT", [D_MODEL, D_MODEL], F16, kind="ExternalInput").ap()
    WkT = nc.dram_tensor("WkT", [D_MODEL, D_MODEL], F16, kind="ExternalInput").ap()
    WvT = nc.dram_tensor("WvT", [D_MODEL, D_MODEL], F16, kind="ExternalInput").ap()
    WoT = nc.dram_tensor("WoT", [D_MODEL, D_MODEL], F16, kind="ExternalInput").ap()
    bq_r = nc.dram_tensor("bq_r", [P, NE], F32, kind="ExternalInput").ap()
    cb_bc = nc.dram_tensor("cb_bc", [P, D_MODEL], F32, kind="ExternalInput").ap()
    out = nc.dram_tensor("out", [LQ, D_MODEL], F32, kind="ExternalOutput").ap()

    with tile.TileContext(nc) as tc:
        with (
            tc.tile_pool(name="persist", bufs=1) as persist,
            tc.tile_pool(name="dram", bufs=1, space="DRAM") as dram,
            tc.tile_pool(name="kt", bufs=8) as kt_pool,     # kT moving tiles
            tc.tile_pool(name="vt", bufs=8) as vt_pool,     # vT stationary tiles
            tc.tile_pool(name="stage", bufs=3) as stage,    # psum->dram staging
            tc.tile_pool(name="vh", bufs=6) as vh_pool,     # AV stationary chunks
            tc.tile_pool(name="exp", bufs=3) as exp_pool,
            tc.tile_pool(name="small", bufs=2) as small,
            tc.tile_pool(name="outst", bufs=2) as outst,
            tc.tile_pool(name="psS", bufs=3, space="PSUM") as psS,  # 6 banks
            tc.tile_pool(name="psV", bufs=2, space="PSUM") as psV,  # 2 banks
        ):
            # ---- persistent SBUF tensors ----
            kpT_sb = persist.tile([P, NE, L], F16)           # kp.T
            qpT_sb = persist.tile([P, NE, LQ], F16)          # qp.T
            attnT_sb = persist.tile([P, NE, LQ], F16)        # normalized attn.T
            qT_sb = persist.tile([P, NE, LQ], F16)
            Wq_sb = persist.tile([P, NE, D_MODEL], F16)
            Wk_sb = persist.tile([P, NE, D_MODEL], F16)
            WvT_sb = persist.tile([P, NE, D_MODEL], F16)
            WoT_sb = persist.tile([P, NE, D_MODEL], F16)
            bq_sb = persist.tile([P, NE], F32)
            cb_sb = persist.tile([P, D_MODEL], F32)

            vp_d = dram.tile([L, NUM_HEADS * HA], F16)       # vp + ones cols

            def load_wT(dst, src):
                nc.sync.dma_start(
                    out=dst[:], in_=src.rearrange("(t p) e -> p t e", p=P))

            # emission order = SP issue order: load what the first matmuls
            # need first; WoT/cb/ones are only needed at the tail.
            nc.sync.dma_start(out=qT_sb[:], in_=qT.rearrange(
                "(t p) l -> p t l", p=P))
            load_wT(Wq_sb, WqT)
            nc.sync.dma_start(out=bq_sb[:], in_=bq_r)
            load_wT(Wk_sb, WkT)
            load_wT(WvT_sb, WvT)

            vp_aug = vp_d[:].rearrange("l (h m) -> l h m", m=HA)

            # ---- qp.T [e, lq] = sum_d WqT[d, e].T @ qT[d, lq], + bq ----
            for e in range(NE):
                ps = psS.tile([P, 1024], F32, name="sc")[:, :512]
                for d in range(NE):
                    nc.tensor.matmul(
                        ps[:, :LQ],
                        Wq_sb[:, d, e * P:(e + 1) * P],
                        qT_sb[:, d, :],
                        start=(d == 0), stop=(d == NE - 1),
                    )
                nc.scalar.activation(
                    qpT_sb[:, e, :], ps[:, :LQ], Act.Identity,
                    bias=bq_sb[:, e:e + 1],
                )

            # ---- K/V projections, interleaved per 1024-wide l group ----
            # kp.T [e, l] (bk dropped: softmax-shift invariant);
            # vp [l, e] with bv folded into cb and a ones column per head.
            for g in range(L // 1024):
                kt_tiles, vt_tiles = [], []
                for d in range(NE):
                    t = kt_pool.tile([P, 1024], F16, tag="kt")
                    nc.sync.dma_start(
                        out=t[:],
                        in_=kT[d * P:(d + 1) * P, g * 1024:(g + 1) * 1024])
                    kt_tiles.append(t)
                    t = vt_pool.tile([P, 1024], F16, tag="vt")
                    nc.sync.dma_start(
                        out=t[:],
                        in_=vT[d * P:(d + 1) * P, g * 1024:(g + 1) * 1024])
                    vt_tiles.append(t)
                for e in range(NE):
                    ps = psS.tile([P, 1024], F32, name="sc")
                    for half in range(2):
                        sl = slice(half * 512, half * 512 + 512)
                        for d in range(NE):
                            nc.tensor.matmul(
                                ps[:, sl],
                                Wk_sb[:, d, e * P:(e + 1) * P],
                                kt_tiles[d][:, sl],
                                start=(d == 0), stop=(d == NE - 1),
                            )
                    nc.vector.tensor_copy(
                        out=kpT_sb[:, e, g * 1024:(g + 1) * 1024], in_=ps[:])
                for lt2 in range(4):
                    st = stage.tile([P, 2, NUM_HEADS, HA], F16, tag="st")
                    for j in range(2):
                        lt_loc = lt2 * 2 + j
                        ps = psS.tile([P, 1024], F32, name="sc")
                        ps1, ps2 = ps[:, 0:512], ps[:, 512:768]
                        for d in range(NE):
                            vsl = vt_tiles[d][:, lt_loc * P:(lt_loc + 1) * P]
                            nc.tensor.matmul(ps1, vsl, WvT_sb[:, d, 0:512],
                                             start=(d == 0), stop=(d == NE - 1))
                            nc.tensor.matmul(ps2, vsl, WvT_sb[:, d, 512:768],
                                             start=(d == 0), stop=(d == NE - 1))
                        nc.vector.tensor_copy(
                            out=st[:, j, :, 0:64],
                            in_=ps[:, :D_MODEL].rearrange(
                                "p (h m) -> p h m", m=64))
                        nc.vector.memset(st[:, j, :, 64:65], 1.0)
                    r0 = g * 1024 + lt2 * 256
                    nc.sync.dma_start(
                        out=vp_d[r0:r0 + 256, :].rearrange(
                            "(a p) m -> p a m", p=P),
                        in_=st[:].rearrange("p a h m -> p a (h m)"))

            load_wT(WoT_sb, WoT)
            nc.sync.dma_start(out=cb_sb[:], in_=cb_bc)

            # ---- attention, head-pair by head-pair ----
            HKT = KT // 2
            for hp in range(NUM_HEADS // 2):
                h0, h1 = 2 * hp, 2 * hp + 1
                et = h0 // 2
                q0 = qpT_sb[0:64, et, :]
                q1 = qpT_sb[64:128, et, :]
                av0 = psV.tile([HA, 512], F32, name="av")
                av1 = psV.tile([HA, 512], F32, name="av")
                vhs = {}
                for half in range(2):
                    for i, h in enumerate((h0, h1)):
                        t = vh_pool.tile([P, HKT, HA], F16, tag="vh")
                        rows = vp_aug[half * HKT * P:(half + 1) * HKT * P, h, :]
                        nc.sync.dma_start(
                            out=t[:], in_=rows.rearrange("(c p) m -> p c m", p=P))
                        vhs[(half, i)] = t
                for c in range(KT):
                    half, cl = c // HKT, c % HKT
                    ps_s = psS.tile([P, 1024], F32, name="sc")
                    nc.tensor.matmul(
                        ps_s[:, 0:LQ],
                        kpT_sb[0:64, et, c * P:(c + 1) * P], q0,
                        start=True, stop=True)
                    nc.tensor.matmul(
                        ps_s[:, 512:512 + LQ],
                        kpT_sb[64:128, et, c * P:(c + 1) * P], q1,
                        start=True, stop=True)
                    ex = exp_pool.tile([P, 1024], F16, tag="exp")
                    nc.scalar.activation(ex[:], ps_s[:], Act.Exp, scale=0.125)
                    nc.tensor.matmul(av0[:, :LQ], vhs[(half, 0)][:, cl, :],
                                     ex[:, 0:LQ],
                                     start=(c == 0), stop=(c == KT - 1))
                    nc.tensor.matmul(av1[:, :LQ], vhs[(half, 1)][:, cl, :],
                                     ex[:, 512:512 + LQ],
                                     start=(c == 0), stop=(c == KT - 1))
                for h, av in ((h0, av0), (h1, av1)):
                    pr = (h % 2) * 64
                    # copy PSUM out immediately so the AV slot frees for the
                    # next pair; the normalize tail works from SBUF
                    av_s = small.tile([HA, 512], F32, tag="avs")
                    nc.vector.tensor_copy(out=av_s[:, :LQ], in_=av[:, :LQ])
                    recip = small.tile([1, 512], F32, tag="recip")
                    nc.vector.reciprocal(out=recip[:, :LQ],
                                         in_=av_s[64:65, :LQ])
                    rbc = small.tile([64, 512], F32, tag="rbc")
                    nc.gpsimd.partition_broadcast(rbc[:, :LQ], recip[:, :LQ])
                    nc.vector.tensor_tensor(
                        out=attnT_sb[pr:pr + 64, et, :],
                        in0=av_s[0:64, :LQ], in1=rbc[:, :LQ],
                        op=mybir.AluOpType.mult,
                    )

            # ---- out[q, e] = attnT.T @ WoT + cb ----
            for qt in range(QT):
                pso = psS.tile([P, 1024], F32, name="sc")
                ps1, ps2 = pso[:, 0:512], pso[:, 512:768]
                for d in range(NE):
                    lhs = attnT_sb[:, d, qt * P:(qt + 1) * P]
                    nc.tensor.matmul(ps1[:], lhs, WoT_sb[:, d, 0:512],
                                     start=(d == 0), stop=(d == NE - 1))
                    nc.tensor.matmul(ps2[:], lhs, WoT_sb[:, d, 512:768],
                                     start=(d == 0), stop=(d == NE - 1))
                ot = outst.tile([P, D_MODEL], F32, tag="ot")
                nc.vector.tensor_tensor(out=ot[:, 0:512], in0=ps1[:],
                                        in1=cb_sb[:, 0:512],
                                        op=mybir.AluOpType.add)
                nc.vector.tensor_tensor(out=ot[:, 512:768], in0=ps2[:],
                                        in1=cb_sb[:, 512:768],
                                        op=mybir.AluOpType.add)
                nc.sync.dma_start(out=out[qt * P:(qt + 1) * P, :], in_=ot[:])

    nc.compile()
    return nc


def make_in_maps(q, k, v, Wq, bq, Wk, bk, Wv, bv, Wo, bo, L, LQ, n_cores):
    f32, f16 = np.float32, np.float16
    qT_full = np.ascontiguousarray(q[0].T, dtype=f16)       # [768, L]
    kT_full = np.ascontiguousarray(k[0].T, dtype=f16)
    vT_full = np.ascontiguousarray(v[0].T, dtype=f16)
    WqT = np.ascontiguousarray(np.asarray(Wq, f32).T.astype(f16))
    WkT = np.ascontiguousarray(np.asarray(Wk, f32).T.astype(f16))
    WvT = np.ascontiguousarray(np.asarray(Wv, f32).T.astype(f16))
    WoT = np.ascontiguousarray(np.asarray(Wo, f32).T.astype(f16))
    bq_r = np.ascontiguousarray(np.asarray(bq, f32).reshape(NE, P).T)
    cb = np.asarray(Wo, f32) @ np.asarray(bv, f32) + np.asarray(bo, f32)
    cb_bc = np.ascontiguousarray(np.broadcast_to(cb, (P, D_MODEL)))
    shared = dict(kT=kT_full, vT=vT_full, WqT=WqT, WkT=WkT, WvT=WvT,
                  WoT=WoT, bq_r=bq_r, cb_bc=cb_bc)
    return [
        {"qT": np.ascontiguousarray(qT_full[:, c * LQ:(c + 1) * LQ]), **shared}
        for c in range(n_cores)
    ]


_PROGRAM_CACHE = {}


def get_program(L, LQ, n_cores):
    key = (L, LQ, n_cores)
    if key not in _PROGRAM_CACHE:
        _PROGRAM_CACHE[key] = build_program(L, LQ, n_cores)
    return _PROGRAM_CACHE[key]


def kernel(q, k, v, Wq, bq, Wk, bk, Wv, bv, Wo, bo):
    B, L, _ = q.shape
    assert B == 1
    n_cores = 8
    LQ = L // n_cores
    nc = get_program(L, LQ, n_cores)
    in_maps = make_in_maps(q, k, v, Wq, bq, Wk, bk, Wv, bv, Wo, bo,
                           L, LQ, n_cores)
    res = run_bass_kernel_spmd(nc, in_maps, core_ids=list(range(n_cores)))
    full = np.concatenate([res.results[c]["out"] for c in range(n_cores)], axis=0)
    return full[None].astype(np.float32)
